# revision 2
# baseline (speedup 1.0000x reference)
"""Causal self-attention (B=4, T=2048, C=1024, H=16) on 8 TRN2 NeuronCores.

Sharding: core = (batch b, head-group g) with b = core//2, g = core%2.
Each core computes, for its batch and its 8 heads:
  QKV projection (W_qkv column shard), causal attention, and a PARTIAL
  output projection (W_pr row shard).  Host sums the two partials per
  batch and adds b_pr.

v2 changes vs baseline (449us):
  - per-chunk K^T / V tiles (no cross-chunk WAR; sweeps of chunk j+1 can
    overlap attention of chunk j so the PE never idles / stays at max
    p-state)
  - diagonal k-tiles narrowed to valid columns (scores/exp/AV skip the
    fully-masked region; affine_select shrinks to the [2,128] diagonal
    sub-block, fused over the head pair via a 2-dim AP)
  - V bias via a precomputed broadcast tile + DVE add (was a PE matmul
    per t-tile)
  - softmax denominators: one fused [1,1024] Ln + Exp(-x) per head-pair
    (was per-head), Dinv broadcast by a rank-1 bf16 matmul into the
    UPPER rows (64:128) of the same yps PSUM bank (tile_position
    (64,64)), one fused [64,1024] PSUM->SBUF copy, then two DVE muls
  - PSUM: ps pool 2x1 bank (sweeps+proj), sc 2x2 banks, yps 1x2 banks
"""

import numpy as np

import concourse.bass as bass
import concourse.mybir as mybir
import concourse.tile as tile
from concourse.bass_utils import run_bass_kernel_spmd


def _split_multiwaits(nc: bass.Bass, max_waits: int = 1) -> None:
    """The walrus build in this container rejects >max_waits sync-waits on an
    instruction ("Too many sync wait commands").  Move extra waits onto
    same-engine NoOps inserted immediately before the instruction — the
    engine blocks on each NoOp's wait first, so semantics are unchanged."""
    n = 0
    for fn in nc.m.functions:
        for blk in fn.blocks:
            out = []
            for inst in blk.instructions:
                si = getattr(inst, "sync_info", None)
                waits = list(si.on_wait) if si is not None and si.on_wait else []
                if len(waits) > max_waits:
                    keep = waits[-max_waits:]
                    for w in waits[: -max_waits]:
                        nop = mybir.InstNoOp(name=f"{inst.name}-w{n}", ins=[], outs=[])
                        n += 1
                        nop.engine = inst.engine
                        nop.sync_info = mybir.SyncInfo(on_wait=[w], on_update=[])
                        out.append(nop)
                    inst.sync_info = mybir.SyncInfo(
                        on_wait=keep, on_update=list(si.on_update or [])
                    )
                out.append(inst)
            blk.instructions = out

AF = mybir.ActivationFunctionType
ALU = mybir.AluOpType

F32 = mybir.dt.float32
BF16 = mybir.dt.bfloat16

B, T_FULL, C = 4, 2048, 1024
H, HD = 16, 64
HPC = 8              # heads per core
GC = HPC * HD        # 512: per-core head-group width
P = 128
CH = 512             # q-chunk width
NKC = C // P         # 8 k-tiles over the C contraction
NTC = CH // P        # 4 t-tiles per chunk

NP_BF16 = mybir.dt.np(BF16)


def build_attention(
    T: int = T_FULL,
    split_waits: bool = True,
    filler_sweep: bool = True,
    filler_proj: bool = True,
    drain_ki: int = 1,
    drain_hp: int = 1,
    debug_taps: bool = False,
) -> bass.Bass:
    assert T % CH == 0
    nch = T // CH        # q-chunks

    nc = bass.Bass("TRN2", debug=False, num_devices=8)

    xT_d = nc.dram_tensor("xT", [C, T], BF16, kind="ExternalInput").ap()
    wq_d = nc.dram_tensor("wq", [C, GC], BF16, kind="ExternalInput").ap()
    wk_d = nc.dram_tensor("wk", [C, GC], BF16, kind="ExternalInput").ap()
    wv_d = nc.dram_tensor("wv", [C, GC], BF16, kind="ExternalInput").ap()
    bq_d = nc.dram_tensor("bq", [GC], F32, kind="ExternalInput").ap()
    bk_d = nc.dram_tensor("bk", [GC], F32, kind="ExternalInput").ap()
    bv_d = nc.dram_tensor("bv", [GC], BF16, kind="ExternalInput").ap()
    wpr_d = nc.dram_tensor("wpr", [GC, C], BF16, kind="ExternalInput").ap()
    out_d = nc.dram_tensor("outT", [C, T], F32, kind="ExternalOutput").ap()
    dbg = {}
    if debug_taps:
        for c in range(T // CH):
            dbg[f"qt{c}"] = nc.dram_tensor(f"dbg_qt{c}", [P, GC // P, CH], BF16, kind="ExternalOutput").ap()
            dbg[f"kt{c}"] = nc.dram_tensor(f"dbg_kt{c}", [P, GC // P, CH], BF16, kind="ExternalOutput").ap()
            dbg[f"v{c}"] = nc.dram_tensor(f"dbg_v{c}", [P, NTC, HPC, HD + 1], BF16, kind="ExternalOutput").ap()
            dbg[f"yt{c}"] = nc.dram_tensor(f"dbg_yt{c}", [P, GC // P, CH], BF16, kind="ExternalOutput").ap()

    with tile.TileContext(nc) as tc:
        with (
            tc.tile_pool(name="singles", bufs=1) as singles,
            tc.tile_pool(name="xt", bufs=2) as xt_pool,
            tc.tile_pool(name="qt", bufs=3) as qt_pool,
            tc.tile_pool(name="at", bufs=6) as at_pool,
            tc.tile_pool(name="yt", bufs=3) as yt_pool,
            tc.tile_pool(name="dd", bufs=2) as d_pool,
            tc.tile_pool(name="bc", bufs=2) as bc_pool,
            tc.tile_pool(name="ot", bufs=3) as out_pool,
            tc.tile_pool(name="ps", bufs=1, space="PSUM") as ps_pool,
            tc.tile_pool(name="bcps", bufs=1, space="PSUM") as bc_psum,
            tc.tile_pool(name="scps", bufs=2, space="PSUM") as sc_ps,
            tc.tile_pool(name="yps", bufs=1, space="PSUM") as y_ps,
        ):
            # ---- resident tensors ----
            # weights arrive in per-output-tile column chunks so the first
            # sweep matmuls can start after ~1/4 of wq instead of the full
            # 4MB of weights
            wq_sb = singles.tile([P, NKC, GC], BF16)
            wk_sb = singles.tile([P, NKC, GC], BF16)
            wv_sb = singles.tile([P, NKC, GC], BF16)
            xt0_t = xt_pool.tile([P, NKC, CH], BF16, tag="xt")
            nc.sync.dma_start(
                out=xt0_t, in_=xT_d[:, 0:CH].rearrange("(kc p) t -> p kc t", p=P)
            )
            for m in range(GC // P):
                msl = slice(m * P, (m + 1) * P)
                nc.sync.dma_start(
                    out=wq_sb[:, :, msl],
                    in_=wq_d[:, msl].rearrange("(kc p) n -> p kc n", p=P),
                )
            for m in range(GC // P):
                msl = slice(m * P, (m + 1) * P)
                nc.sync.dma_start(
                    out=wk_sb[:, :, msl],
                    in_=wk_d[:, msl].rearrange("(kc p) n -> p kc n", p=P),
                )
            nc.sync.dma_start(out=wv_sb, in_=wv_d.rearrange("(kc p) n -> p kc n", p=P))
            wpr_sb = singles.tile([P, GC // P, C], BF16)
            nc.sync.dma_start(
                out=wpr_sb, in_=wpr_d.rearrange("(kp p) m -> p kp m", p=P)
            )
            bqk_sb = singles.tile([P, 2, GC // P], F32)
            nc.sync.dma_start(
                out=bqk_sb[:, 0, :], in_=bq_d.rearrange("(m p) -> p m", p=P)
            )
            nc.sync.dma_start(
                out=bqk_sb[:, 1, :], in_=bk_d.rearrange("(m p) -> p m", p=P)
            )
            bv_sb = singles.tile([1, GC], BF16)
            nc.sync.dma_start(out=bv_sb, in_=bv_d.rearrange("(o n) -> o n", o=1))
            ones_sb = singles.tile([1, P], BF16)
            nc.vector.memset(ones_sb, 1.0)
            ones64_sb = singles.tile([P, HD], BF16)
            nc.vector.memset(ones64_sb, 1.0)

            # V bias as a [128, GC] broadcast tile (built once via rank-1 mm)
            bvb_ps = ps_pool.tile([P, GC], F32, tag="ps")
            nc.tensor.matmul(bvb_ps, lhsT=ones_sb, rhs=bv_sb, start=True, stop=True)
            bvb_sb = singles.tile([P, GC], BF16)
            nc.vector.tensor_copy(bvb_sb, bvb_ps)

            # causal triangle mask for the [128,128] diagonal sub-block,
            # doubled along a middle dim so one DVE mul covers both heads
            # of a pair: keep (p, f) iff f >= p
            tri_sb = singles.tile([P, 2, P], BF16)
            nc.vector.memset(tri_sb, 1.0)
            nc.gpsimd.affine_select(
                out=tri_sb,
                in_=tri_sb,
                pattern=[[0, 2], [1, P]],
                compare_op=ALU.is_ge,
                fill=0.0,
                base=0,
                channel_multiplier=-1,
            )

            # per-chunk K^T and V tiles (separate tiles -> no cross-chunk WAR)
            kt_cs = [
                singles.tile([P, GC // P, CH], BF16, name=f"kt{c}") for c in range(nch)
            ]
            v_cs = [
                singles.tile([P, NTC, HPC, HD + 1], BF16, name=f"v{c}")
                for c in range(nch)
            ]
            for c in range(nch):
                nc.vector.memset(v_cs[c][:, :, :, HD : HD + 1], 1.0)

            # ---- filler queue: sweep/projection work emitted interleaved
            # with attention iterations so the PE instruction stream stays
            # dense (keeps the PE p-state ramped and fills exp-wait gaps) ----
            from collections import deque

            filler_q = deque()
            qts = {}

            def drain(n):
                for _ in range(min(n, len(filler_q))):
                    filler_q.popleft()()

            def drain_all():
                while filler_q:
                    filler_q.popleft()()

            def enqueue_sweep(jj):
                """Queue QKV-sweep emission units for chunk jj (1 matmul or
                1 DVE add per unit). Issues the xt DMA immediately."""
                if jj == 0:
                    xt_t = xt0_t
                else:
                    xt_t = xt_pool.tile(
                        [P, NKC, CH], BF16, tag="xt", name=f"xt{jj}"
                    )

                    def xt_dma():
                        nc.sync.dma_start(
                            out=xt_t,
                            in_=xT_d[:, jj * CH : (jj + 1) * CH].rearrange(
                                "(kc p) t -> p kc t", p=P
                            ),
                        )

                    filler_q.append(xt_dma)
                qt_t = qt_pool.tile([P, GC // P, CH], BF16, tag="qt", name=f"qt{jj}")
                qts[jj] = qt_t
                box = {}

                def q_mm(mq, kc):
                    if kc == 0:
                        box["ps"] = ps_pool.tile(
                            [P, CH], F32, tag="ps", name=f"psq{jj}_{mq}"
                        )
                    nc.tensor.matmul(
                        box["ps"],
                        lhsT=wq_sb[:, kc, mq * P : (mq + 1) * P],
                        rhs=xt_t[:, kc, :],
                        start=(kc == 0),
                        stop=(kc == NKC - 1),
                    )

                def q_add(mq):
                    nc.vector.tensor_scalar_add(
                        out=qt_t[:, mq, :],
                        in0=box["ps"],
                        scalar1=bqk_sb[:, 0, mq : mq + 1],
                    )

                def k_mm(mk, kc):
                    if kc == 0:
                        box["ps"] = ps_pool.tile(
                            [P, CH], F32, tag="ps", name=f"psk{jj}_{mk}"
                        )
                    nc.tensor.matmul(
                        box["ps"],
                        lhsT=wk_sb[:, kc, mk * P : (mk + 1) * P],
                        rhs=xt_t[:, kc, :],
                        start=(kc == 0),
                        stop=(kc == NKC - 1),
                    )

                def k_add(mk):
                    nc.vector.tensor_scalar_add(
                        out=kt_cs[jj][:, mk, :],
                        in0=box["ps"],
                        scalar1=bqk_sb[:, 1, mk : mk + 1],
                    )

                def v_mm(tl, kc):
                    if kc == 0:
                        box["ps"] = ps_pool.tile(
                            [P, GC], F32, tag="ps", name=f"psv{jj}_{tl}"
                        )
                    nc.tensor.matmul(
                        box["ps"],
                        lhsT=xt_t[:, kc, tl * P : (tl + 1) * P],
                        rhs=wv_sb[:, kc, :],
                        start=(kc == 0),
                        stop=(kc == NKC - 1),
                    )

                def v_add(tl):
                    nc.vector.tensor_add(
                        v_cs[jj][:, tl, :, 0:HD],
                        box["ps"].rearrange("p (h d) -> p h d", h=HPC),
                        bvb_sb.rearrange("p (h d) -> p h d", h=HPC),
                    )

                def q_tile(mq):
                    for kc in range(NKC):
                        q_mm(mq, kc)
                    q_add(mq)

                def k_tile(mk):
                    for kc in range(NKC):
                        k_mm(mk, kc)
                    k_add(mk)

                def v_tile(tl):
                    for kc in range(NKC):
                        v_mm(tl, kc)
                    v_add(tl)

                for mq in range(GC // P):
                    filler_q.append(lambda mq=mq: q_tile(mq))
                for mk in range(GC // P):
                    filler_q.append(lambda mk=mk: k_tile(mk))
                for tl in range(NTC):
                    filler_q.append(lambda tl=tl: v_tile(tl))

            def enqueue_proj(jj, yt_t):
                box = {}

                def p_mm(m, kp):
                    if kp == 0:
                        box["pp"] = ps_pool.tile(
                            [P, CH], F32, tag="ps", name=f"pp{jj}_{m}"
                        )
                    nc.tensor.matmul(
                        box["pp"],
                        lhsT=wpr_sb[:, kp, m * P : (m + 1) * P],
                        rhs=yt_t[:, kp, :],
                        start=(kp == 0),
                        stop=(kp == GC // P - 1),
                    )

                def p_out(m):
                    ot = out_pool.tile([P, CH], F32, tag="ot", name=f"ot{jj}_{m}")
                    nc.vector.tensor_copy(ot, box["pp"])
                    nc.sync.dma_start(
                        out=out_d[m * P : (m + 1) * P, jj * CH : (jj + 1) * CH],
                        in_=ot,
                    )

                def p_tile(m):
                    for kp in range(GC // P):
                        p_mm(m, kp)
                    p_out(m)

                for m in range(C // P):
                    filler_q.append(lambda m=m: p_tile(m))

            enqueue_sweep(0)
            drain_all()

            for j in range(nch):
                nkt = NTC * (j + 1)         # k-tiles valid for this q-chunk
                if j + 1 < nch:
                    enqueue_sweep(j + 1)
                    if not filler_sweep:
                        drain_all()
                qt_t = qts[j]

                # ---- attention for chunk j ----
                yt_t = yt_pool.tile([P, GC // P, CH], BF16, tag="yt", name=f"yt{j}")
                for hp in range(HPC // 2):
                    yps = y_ps.tile([P, 2 * CH], F32, tag="yps", name=f"yps{j}_{hp}")
                    for ki in range(nkt):
                        c, tl = divmod(ki, NTC)
                        r = ki - NTC * j
                        off = 0 if r < 0 else P * r
                        sc = sc_ps.tile(
                            [P, 2 * CH], F32, tag="sc", name=f"sc{j}_{hp}_{ki}"
                        )
                        scv = sc.rearrange("p (u f) -> p u f", u=2)
                        for u in range(2):  # u = head parity; po = 64*u
                            po = HD * u
                            nc.tensor.matmul(
                                scv[:, u, off:],
                                lhsT=kt_cs[c][po : po + HD, hp, tl * P : (tl + 1) * P],
                                rhs=qt_t[po : po + HD, hp, off:],
                                start=True,
                                stop=True,
                            )
                        at_t = at_pool.tile([P, 2 * CH], BF16)
                        atv = at_t.rearrange("p (u f) -> p u f", u=2)
                        nc.scalar.activation(
                            atv[:, :, off:], scv[:, :, off:], AF.Exp, scale=0.125
                        )
                        if r >= 0:  # diagonal sub-block: zero where q < k
                            nc.vector.tensor_mul(
                                atv[:, :, off : off + P],
                                atv[:, :, off : off + P],
                                tri_sb,
                            )
                        for u in range(2):
                            nc.tensor.matmul(
                                yps[0 : HD + 1, u * CH + off : (u + 1) * CH],
                                lhsT=v_cs[c][:, tl, 2 * hp + u, :],
                                rhs=atv[:, u, off:],
                                start=(ki == 0),
                                stop=(ki == nkt - 1),
                                skip_group_check=True,
                            )
                        drain(drain_ki)
                    # ---- drain yps to SBUF at once (frees the PSUM slot for
                    # the next head pair); normalize off the critical path:
                    # Dinv = exp(-ln D) fused over the pair, broadcast via
                    # rank-1 bf16 matmuls into 1-bank ps tiles, then two DVE
                    # muls reading yraw (SBUF) x bc (PSUM) ----
                    yraw = bc_pool.tile([P, 2 * CH], BF16, tag="yraw")
                    nc.vector.tensor_copy(yraw[0 : HD + 1, :], yps[0 : HD + 1, :])
                    dr_t = d_pool.tile([P, 2 * CH], F32)
                    dr2_t = d_pool.tile([P, 2 * CH], BF16, tag="dr2")
                    nc.scalar.activation(
                        dr_t[HD : HD + 1, :], yraw[HD : HD + 1, :], AF.Ln
                    )
                    nc.scalar.activation(
                        dr2_t[HD : HD + 1, :],
                        dr_t[HD : HD + 1, :],
                        AF.Exp,
                        scale=-1.0,
                    )
                    for u in range(2):
                        po = HD * u
                        bc_ps = bc_psum.tile([HD, CH], F32, tag="bcps")
                        nc.tensor.matmul(
                            bc_ps,
                            lhsT=ones64_sb[HD : HD + 1, :],
                            rhs=dr2_t[HD : HD + 1, u * CH : (u + 1) * CH],
                            start=True,
                            stop=True,
                            tile_position=(HD, 0),
                        )
                        nc.vector.tensor_mul(
                            yt_t[po : po + HD, hp, :],
                            yraw[0:HD, u * CH : (u + 1) * CH],
                            bc_ps,
                        )
                    drain(drain_hp)

                if debug_taps:
                    nc.sync.dma_start(out=dbg[f"qt{j}"], in_=qt_t)
                    nc.sync.dma_start(out=dbg[f"kt{j}"], in_=kt_cs[j])
                    nc.sync.dma_start(out=dbg[f"v{j}"], in_=v_cs[j])
                    nc.sync.dma_start(out=dbg[f"yt{j}"], in_=yt_t)

                # ---- partial output projection for chunk j, as filler ----
                enqueue_proj(j, yt_t)
                if not filler_proj:
                    drain_all()

            drain_all()

    if split_waits:  # breaks CoreSim's sem bookkeeping; needed for walrus
        _split_multiwaits(nc)
    return nc


def make_in_maps(x, W_qkv, b_qkv, W_pr):
    """Shard FULL inputs into the 8 per-core input dicts."""
    x = np.asarray(x, dtype=np.float32)
    W_qkv = np.asarray(W_qkv, dtype=np.float32)
    b_qkv = np.asarray(b_qkv, dtype=np.float32)
    W_pr = np.asarray(W_pr, dtype=np.float32)
    in_maps = []
    for core in range(8):
        b, g = divmod(core, 2)
        sl = slice(g * GC, (g + 1) * GC)
        in_maps.append(
            {
                "xT": np.ascontiguousarray(x[b].T).astype(NP_BF16),
                "wq": np.ascontiguousarray(W_qkv[:, 0 * C :][:, sl]).astype(NP_BF16),
                "wk": np.ascontiguousarray(W_qkv[:, 1 * C :][:, sl]).astype(NP_BF16),
                "wv": np.ascontiguousarray(W_qkv[:, 2 * C :][:, sl]).astype(NP_BF16),
                "bq": np.ascontiguousarray(b_qkv[0 * C :][sl]),
                "bk": np.ascontiguousarray(b_qkv[1 * C :][sl]),
                "bv": np.ascontiguousarray(b_qkv[2 * C :][sl]).astype(NP_BF16),
                "wpr": np.ascontiguousarray(W_pr[sl, :]).astype(NP_BF16),
            }
        )
    return in_maps


def assemble_output(parts, b_pr):
    """parts: 8 per-core outT [C, T] partials -> full [B, T, C] output."""
    b_pr = np.asarray(b_pr, dtype=np.float32)
    out = np.empty((B, T_FULL, C), dtype=np.float32)
    for b in range(B):
        out[b] = (parts[2 * b] + parts[2 * b + 1]).T + b_pr
    return out


_CACHE = {}


def kernel(x, W_qkv, b_qkv, W_pr, b_pr):
    if "nc" not in _CACHE:
        _CACHE["nc"] = build_attention(T_FULL)
    in_maps = make_in_maps(x, W_qkv, b_qkv, W_pr)
    res = run_bass_kernel_spmd(_CACHE["nc"], in_maps, core_ids=list(range(8)))
    parts = [r["outT"] for r in res.results]
    return assemble_output(parts, b_pr)


# revision 3
# speedup vs baseline: 1.0277x; 1.0277x over previous
"""Causal self-attention (B=4, T=2048, C=1024, H=16) on 8 TRN2 NeuronCores.

Sharding: core = (batch b, head-group g) with b = core//2, g = core%2.
Each core computes, for its batch and its 8 heads:
  QKV projection (W_qkv column shard), causal attention, and a PARTIAL
  output projection (W_pr row shard).  Host sums the two partials per
  batch and adds b_pr.

v2 changes vs baseline (449us):
  - per-chunk K^T / V tiles (no cross-chunk WAR; sweeps of chunk j+1 can
    overlap attention of chunk j so the PE never idles / stays at max
    p-state)
  - diagonal k-tiles narrowed to valid columns (scores/exp/AV skip the
    fully-masked region; affine_select shrinks to the [2,128] diagonal
    sub-block, fused over the head pair via a 2-dim AP)
  - V bias via a precomputed broadcast tile + DVE add (was a PE matmul
    per t-tile)
  - softmax denominators: one fused [1,1024] Ln + Exp(-x) per head-pair
    (was per-head), Dinv broadcast by a rank-1 bf16 matmul into the
    UPPER rows (64:128) of the same yps PSUM bank (tile_position
    (64,64)), one fused [64,1024] PSUM->SBUF copy, then two DVE muls
  - PSUM: ps pool 2x1 bank (sweeps+proj), sc 2x2 banks, yps 1x2 banks
"""

import numpy as np

import concourse.bass as bass
import concourse.mybir as mybir
import concourse.tile as tile
from concourse.bass_utils import run_bass_kernel_spmd


def _split_multiwaits(nc: bass.Bass, max_waits: int = 1) -> None:
    """The walrus build in this container rejects >max_waits sync-waits on an
    instruction ("Too many sync wait commands").  Move extra waits onto
    same-engine NoOps inserted immediately before the instruction — the
    engine blocks on each NoOp's wait first, so semantics are unchanged."""
    n = 0
    for fn in nc.m.functions:
        for blk in fn.blocks:
            out = []
            for inst in blk.instructions:
                si = getattr(inst, "sync_info", None)
                waits = list(si.on_wait) if si is not None and si.on_wait else []
                if len(waits) > max_waits:
                    keep = waits[-max_waits:]
                    for w in waits[: -max_waits]:
                        nop = mybir.InstNoOp(name=f"{inst.name}-w{n}", ins=[], outs=[])
                        n += 1
                        nop.engine = inst.engine
                        nop.sync_info = mybir.SyncInfo(on_wait=[w], on_update=[])
                        out.append(nop)
                    inst.sync_info = mybir.SyncInfo(
                        on_wait=keep, on_update=list(si.on_update or [])
                    )
                out.append(inst)
            blk.instructions = out

AF = mybir.ActivationFunctionType
ALU = mybir.AluOpType

F32 = mybir.dt.float32
BF16 = mybir.dt.bfloat16

B, T_FULL, C = 4, 2048, 1024
H, HD = 16, 64
HPC = 8              # heads per core
GC = HPC * HD        # 512: per-core head-group width
P = 128
CH = 512             # q-chunk width
NKC = C // P         # 8 k-tiles over the C contraction
NTC = CH // P        # 4 t-tiles per chunk

NP_BF16 = mybir.dt.np(BF16)


def build_attention(
    T: int = T_FULL,
    split_waits: bool = True,
    filler_sweep: bool = True,
    filler_proj: bool = True,
    drain_ki: int = 1,
    drain_hp: int = 1,
    debug_taps: bool = False,
) -> bass.Bass:
    assert T % CH == 0
    nch = T // CH        # q-chunks

    nc = bass.Bass("TRN2", debug=False, num_devices=8)

    xT_d = nc.dram_tensor("xT", [C, T], BF16, kind="ExternalInput").ap()
    wq_d = nc.dram_tensor("wq", [C, GC], BF16, kind="ExternalInput").ap()
    wk_d = nc.dram_tensor("wk", [C, GC], BF16, kind="ExternalInput").ap()
    wv_d = nc.dram_tensor("wv", [C, GC], BF16, kind="ExternalInput").ap()
    bq_d = nc.dram_tensor("bq", [GC], F32, kind="ExternalInput").ap()
    bk_d = nc.dram_tensor("bk", [GC], F32, kind="ExternalInput").ap()
    bv_d = nc.dram_tensor("bv", [GC], BF16, kind="ExternalInput").ap()
    wpr_d = nc.dram_tensor("wpr", [GC, C], BF16, kind="ExternalInput").ap()
    out_d = nc.dram_tensor("outT", [C, T], F32, kind="ExternalOutput").ap()
    dbg = {}
    if debug_taps:
        for c in range(T // CH):
            dbg[f"qt{c}"] = nc.dram_tensor(f"dbg_qt{c}", [P, GC // P, CH], BF16, kind="ExternalOutput").ap()
            dbg[f"kt{c}"] = nc.dram_tensor(f"dbg_kt{c}", [P, GC // P, CH], BF16, kind="ExternalOutput").ap()
            dbg[f"v{c}"] = nc.dram_tensor(f"dbg_v{c}", [P, NTC, HPC, HD + 1], BF16, kind="ExternalOutput").ap()
            dbg[f"yt{c}"] = nc.dram_tensor(f"dbg_yt{c}", [P, GC // P, CH], BF16, kind="ExternalOutput").ap()

    with tile.TileContext(nc) as tc:
        with (
            tc.tile_pool(name="singles", bufs=1) as singles,
            tc.tile_pool(name="xt", bufs=2) as xt_pool,
            tc.tile_pool(name="qt", bufs=3) as qt_pool,
            tc.tile_pool(name="at", bufs=6) as at_pool,
            tc.tile_pool(name="yt", bufs=3) as yt_pool,
            tc.tile_pool(name="dd", bufs=2) as d_pool,
            tc.tile_pool(name="bc", bufs=2) as bc_pool,
            tc.tile_pool(name="ot", bufs=3) as out_pool,
            tc.tile_pool(name="ps", bufs=1, space="PSUM") as ps_pool,
            tc.tile_pool(name="bcps", bufs=1, space="PSUM") as bc_psum,
            tc.tile_pool(name="scps", bufs=2, space="PSUM") as sc_ps,
            tc.tile_pool(name="yps", bufs=1, space="PSUM") as y_ps,
        ):
            # ---- resident tensors ----
            # weights arrive in per-output-tile column chunks so the first
            # sweep matmuls can start after ~1/4 of wq instead of the full
            # 4MB of weights
            wq_sb = singles.tile([P, NKC, GC], BF16)
            wk_sb = singles.tile([P, NKC, GC], BF16)
            wv_sb = singles.tile([P, NKC, GC], BF16)
            xt0_t = xt_pool.tile([P, NKC, CH], BF16, tag="xt")
            for kc in range(NKC):
                nc.sync.dma_start(
                    out=xt0_t[:, kc, :], in_=xT_d[kc * P : (kc + 1) * P, 0:CH]
                )
            for m in range(GC // P):
                msl = slice(m * P, (m + 1) * P)
                nc.sync.dma_start(
                    out=wq_sb[:, :, msl],
                    in_=wq_d[:, msl].rearrange("(kc p) n -> p kc n", p=P),
                )
            for m in range(GC // P):
                msl = slice(m * P, (m + 1) * P)
                nc.sync.dma_start(
                    out=wk_sb[:, :, msl],
                    in_=wk_d[:, msl].rearrange("(kc p) n -> p kc n", p=P),
                )
            nc.sync.dma_start(out=wv_sb, in_=wv_d.rearrange("(kc p) n -> p kc n", p=P))
            wpr_sb = singles.tile([P, GC // P, C], BF16)
            nc.sync.dma_start(
                out=wpr_sb, in_=wpr_d.rearrange("(kp p) m -> p kp m", p=P)
            )
            bqk_sb = singles.tile([P, 2, GC // P], F32)
            nc.sync.dma_start(
                out=bqk_sb[:, 0, :], in_=bq_d.rearrange("(m p) -> p m", p=P)
            )
            nc.sync.dma_start(
                out=bqk_sb[:, 1, :], in_=bk_d.rearrange("(m p) -> p m", p=P)
            )
            bv_sb = singles.tile([1, GC], BF16)
            nc.sync.dma_start(out=bv_sb, in_=bv_d.rearrange("(o n) -> o n", o=1))
            ones_sb = singles.tile([1, P], BF16)
            nc.vector.memset(ones_sb, 1.0)
            ones64_sb = singles.tile([P, HD], BF16)
            nc.vector.memset(ones64_sb, 1.0)

            # V bias as a [128, GC] broadcast tile (built once via rank-1 mm)
            bvb_ps = ps_pool.tile([P, GC], F32, tag="ps")
            nc.tensor.matmul(bvb_ps, lhsT=ones_sb, rhs=bv_sb, start=True, stop=True)
            bvb_sb = singles.tile([P, GC], BF16)
            nc.vector.tensor_copy(bvb_sb, bvb_ps)

            # causal triangle mask for the [128,128] diagonal sub-block,
            # doubled along a middle dim so one DVE mul covers both heads
            # of a pair: keep (p, f) iff f >= p
            tri_sb = singles.tile([P, 2, P], BF16)
            nc.vector.memset(tri_sb, 1.0)
            nc.gpsimd.affine_select(
                out=tri_sb,
                in_=tri_sb,
                pattern=[[0, 2], [1, P]],
                compare_op=ALU.is_ge,
                fill=0.0,
                base=0,
                channel_multiplier=-1,
            )

            # per-chunk K^T and V tiles (separate tiles -> no cross-chunk WAR)
            kt_cs = [
                singles.tile([P, GC // P, CH], BF16, name=f"kt{c}") for c in range(nch)
            ]
            v_cs = [
                singles.tile([P, NTC, HPC, HD + 1], BF16, name=f"v{c}")
                for c in range(nch)
            ]
            for c in range(nch):
                nc.vector.memset(v_cs[c][:, :, :, HD : HD + 1], 1.0)

            # ---- filler queue: sweep/projection work emitted interleaved
            # with attention iterations so the PE instruction stream stays
            # dense (keeps the PE p-state ramped and fills exp-wait gaps) ----
            from collections import deque

            filler_q = deque()
            qts = {}
            counts = {"enq": 0, "dr": 0}
            need = {}

            def enq(fn):
                filler_q.append(fn)
                counts["enq"] += 1

            def drain1():
                if filler_q:
                    filler_q.popleft()()
                    counts["dr"] += 1

            def drain_all():
                while filler_q:
                    drain1()

            def enqueue_sweep(jj):
                """Queue QKV-sweep emission units for chunk jj (1 matmul or
                1 DVE add per unit). Issues the xt DMA immediately."""
                if jj == 0:
                    xt_t = xt0_t
                else:
                    xt_t = xt_pool.tile(
                        [P, NKC, CH], BF16, tag="xt", name=f"xt{jj}"
                    )

                    def xt_dma():
                        nc.sync.dma_start(
                            out=xt_t,
                            in_=xT_d[:, jj * CH : (jj + 1) * CH].rearrange(
                                "(kc p) t -> p kc t", p=P
                            ),
                        )

                    enq(xt_dma)
                qt_t = qt_pool.tile([P, GC // P, CH], BF16, tag="qt", name=f"qt{jj}")
                qts[jj] = qt_t
                box = {}

                def swps(nm):
                    if jj == 0 and swps.n % 2 == 1:
                        t = sc_ps.tile([P, CH], F32, tag="sc", name=nm)
                    else:
                        t = ps_pool.tile([P, CH], F32, tag="ps", name=nm)
                    swps.n += 1
                    return t

                swps.n = 0

                def q_mm(mq, kc):
                    if kc == 0:
                        box["ps"] = swps(f"psq{jj}_{mq}")
                    nc.tensor.matmul(
                        box["ps"],
                        lhsT=wq_sb[:, kc, mq * P : (mq + 1) * P],
                        rhs=xt_t[:, kc, :],
                        start=(kc == 0),
                        stop=(kc == NKC - 1),
                    )

                def q_add(mq):
                    nc.vector.tensor_scalar_add(
                        out=qt_t[:, mq, :],
                        in0=box["ps"],
                        scalar1=bqk_sb[:, 0, mq : mq + 1],
                    )

                def k_mm(mk, kc):
                    if kc == 0:
                        box["ps"] = swps(f"psk{jj}_{mk}")
                    nc.tensor.matmul(
                        box["ps"],
                        lhsT=wk_sb[:, kc, mk * P : (mk + 1) * P],
                        rhs=xt_t[:, kc, :],
                        start=(kc == 0),
                        stop=(kc == NKC - 1),
                    )

                def k_add(mk):
                    nc.vector.tensor_scalar_add(
                        out=kt_cs[jj][:, mk, :],
                        in0=box["ps"],
                        scalar1=bqk_sb[:, 1, mk : mk + 1],
                    )

                def v_mm(tl, kc):
                    if kc == 0:
                        box["ps"] = swps(f"psv{jj}_{tl}")
                    nc.tensor.matmul(
                        box["ps"],
                        lhsT=xt_t[:, kc, tl * P : (tl + 1) * P],
                        rhs=wv_sb[:, kc, :],
                        start=(kc == 0),
                        stop=(kc == NKC - 1),
                    )

                def v_add(tl):
                    nc.vector.tensor_add(
                        v_cs[jj][:, tl, :, 0:HD],
                        box["ps"].rearrange("p (h d) -> p h d", h=HPC),
                        bvb_sb.rearrange("p (h d) -> p h d", h=HPC),
                    )

                def q_tile(mq):
                    for kc in range(NKC):
                        q_mm(mq, kc)
                    q_add(mq)

                def k_tile(mk):
                    for kc in range(NKC):
                        k_mm(mk, kc)
                    k_add(mk)

                def v_tile(tl):
                    for kc in range(NKC):
                        v_mm(tl, kc)
                    v_add(tl)

                for mq in range(GC // P):
                    enq(lambda mq=mq: q_tile(mq))
                for mk in range(GC // P):
                    enq(lambda mk=mk: k_tile(mk))
                for tl in range(NTC):
                    enq(lambda tl=tl: v_tile(tl))

            def enqueue_proj(jj, yt_t):
                box = {}

                def p_mm(m, kp):
                    if kp == 0:
                        box["pp"] = ps_pool.tile(
                            [P, CH], F32, tag="ps", name=f"pp{jj}_{m}"
                        )
                    nc.tensor.matmul(
                        box["pp"],
                        lhsT=wpr_sb[:, kp, m * P : (m + 1) * P],
                        rhs=yt_t[:, kp, :],
                        start=(kp == 0),
                        stop=(kp == GC // P - 1),
                    )

                def p_out(m):
                    ot = out_pool.tile([P, CH], F32, tag="ot", name=f"ot{jj}_{m}")
                    nc.vector.tensor_copy(ot, box["pp"])
                    nc.sync.dma_start(
                        out=out_d[m * P : (m + 1) * P, jj * CH : (jj + 1) * CH],
                        in_=ot,
                    )

                def p_tile(m):
                    for kp in range(GC // P):
                        p_mm(m, kp)
                    p_out(m)

                for m in range(C // P):
                    enq(lambda m=m: p_tile(m))

            enqueue_sweep(0)
            drain_all()

            for j in range(nch):
                nkt = NTC * (j + 1)         # k-tiles valid for this q-chunk
                if j + 1 < nch:
                    enqueue_sweep(j + 1)
                    need[j + 1] = counts["enq"]
                    if not filler_sweep:
                        drain_all()
                # sweep j must be fully EMITTED before attention j reads it
                while counts["dr"] < need.get(j, 0):
                    drain1()
                # spread the present queue evenly over this chunk's iterations
                dr_base = counts["dr"]
                q_now = len(filler_q)
                iters = nkt * (HPC // 2)
                it_i = 0
                qt_t = qts[j]

                # ---- attention for chunk j ----
                yt_t = yt_pool.tile([P, GC // P, CH], BF16, tag="yt", name=f"yt{j}")
                for hp in range(HPC // 2):
                    yps = y_ps.tile([P, 2 * CH], F32, tag="yps", name=f"yps{j}_{hp}")
                    for ki in range(nkt):
                        c, tl = divmod(ki, NTC)
                        r = ki - NTC * j
                        off = 0 if r < 0 else P * r
                        sc = sc_ps.tile(
                            [P, 2 * CH], F32, tag="sc", name=f"sc{j}_{hp}_{ki}"
                        )
                        scv = sc.rearrange("p (u f) -> p u f", u=2)
                        for u in range(2):  # u = head parity; po = 64*u
                            po = HD * u
                            nc.tensor.matmul(
                                scv[:, u, off:],
                                lhsT=kt_cs[c][po : po + HD, hp, tl * P : (tl + 1) * P],
                                rhs=qt_t[po : po + HD, hp, off:],
                                start=True,
                                stop=True,
                            )
                        at_t = at_pool.tile([P, 2 * CH], BF16)
                        atv = at_t.rearrange("p (u f) -> p u f", u=2)
                        nc.scalar.activation(
                            atv[:, :, off:], scv[:, :, off:], AF.Exp, scale=0.125
                        )
                        if r >= 0:  # diagonal sub-block: zero where q < k
                            nc.vector.tensor_mul(
                                atv[:, :, off : off + P],
                                atv[:, :, off : off + P],
                                tri_sb,
                            )
                        for u in range(2):
                            nc.tensor.matmul(
                                yps[0 : HD + 1, u * CH + off : (u + 1) * CH],
                                lhsT=v_cs[c][:, tl, 2 * hp + u, :],
                                rhs=atv[:, u, off:],
                                start=(ki == 0),
                                stop=(ki == nkt - 1),
                                skip_group_check=True,
                            )
                        it_i += 1
                        while (
                            counts["dr"] - dr_base < (it_i * q_now) // iters
                            and filler_q
                        ):
                            drain1()
                    # ---- drain yps to SBUF at once (frees the PSUM slot for
                    # the next head pair); normalize off the critical path:
                    # Dinv = exp(-ln D) fused over the pair, broadcast via
                    # rank-1 bf16 matmuls into 1-bank ps tiles, then two DVE
                    # muls reading yraw (SBUF) x bc (PSUM) ----
                    yraw = bc_pool.tile([P, 2 * CH], BF16, tag="yraw")
                    nc.vector.tensor_copy(yraw[0 : HD + 1, :], yps[0 : HD + 1, :])
                    dr_t = d_pool.tile([P, 2 * CH], F32)
                    dr2_t = d_pool.tile([P, 2 * CH], BF16, tag="dr2")
                    nc.scalar.activation(
                        dr_t[HD : HD + 1, :], yraw[HD : HD + 1, :], AF.Ln
                    )
                    nc.scalar.activation(
                        dr2_t[HD : HD + 1, :],
                        dr_t[HD : HD + 1, :],
                        AF.Exp,
                        scale=-1.0,
                    )
                    for u in range(2):
                        po = HD * u
                        bc_ps = bc_psum.tile([HD, CH], F32, tag="bcps")
                        nc.tensor.matmul(
                            bc_ps,
                            lhsT=ones64_sb[HD : HD + 1, :],
                            rhs=dr2_t[HD : HD + 1, u * CH : (u + 1) * CH],
                            start=True,
                            stop=True,
                            tile_position=(HD, 0),
                        )
                        nc.vector.tensor_mul(
                            yt_t[po : po + HD, hp, :],
                            yraw[0:HD, u * CH : (u + 1) * CH],
                            bc_ps,
                        )

                if debug_taps:
                    nc.sync.dma_start(out=dbg[f"qt{j}"], in_=qt_t)
                    nc.sync.dma_start(out=dbg[f"kt{j}"], in_=kt_cs[j])
                    nc.sync.dma_start(out=dbg[f"v{j}"], in_=v_cs[j])
                    nc.sync.dma_start(out=dbg[f"yt{j}"], in_=yt_t)

                # ---- partial output projection for chunk j, as filler ----
                enqueue_proj(j, yt_t)
                if not filler_proj:
                    drain_all()

            drain_all()

    if split_waits:  # breaks CoreSim's sem bookkeeping; needed for walrus
        _split_multiwaits(nc)
    return nc


def make_in_maps(x, W_qkv, b_qkv, W_pr):
    """Shard FULL inputs into the 8 per-core input dicts."""
    x = np.asarray(x, dtype=np.float32)
    W_qkv = np.asarray(W_qkv, dtype=np.float32)
    b_qkv = np.asarray(b_qkv, dtype=np.float32)
    W_pr = np.asarray(W_pr, dtype=np.float32)
    in_maps = []
    for core in range(8):
        b, g = divmod(core, 2)
        sl = slice(g * GC, (g + 1) * GC)
        in_maps.append(
            {
                "xT": np.ascontiguousarray(x[b].T).astype(NP_BF16),
                "wq": np.ascontiguousarray(W_qkv[:, 0 * C :][:, sl]).astype(NP_BF16),
                "wk": np.ascontiguousarray(W_qkv[:, 1 * C :][:, sl]).astype(NP_BF16),
                "wv": np.ascontiguousarray(W_qkv[:, 2 * C :][:, sl]).astype(NP_BF16),
                "bq": np.ascontiguousarray(b_qkv[0 * C :][sl]),
                "bk": np.ascontiguousarray(b_qkv[1 * C :][sl]),
                "bv": np.ascontiguousarray(b_qkv[2 * C :][sl]).astype(NP_BF16),
                "wpr": np.ascontiguousarray(W_pr[sl, :]).astype(NP_BF16),
            }
        )
    return in_maps


def assemble_output(parts, b_pr):
    """parts: 8 per-core outT [C, T] partials -> full [B, T, C] output."""
    b_pr = np.asarray(b_pr, dtype=np.float32)
    out = np.empty((B, T_FULL, C), dtype=np.float32)
    for b in range(B):
        out[b] = (parts[2 * b] + parts[2 * b + 1]).T + b_pr
    return out


_CACHE = {}


def kernel(x, W_qkv, b_qkv, W_pr, b_pr):
    if "nc" not in _CACHE:
        _CACHE["nc"] = build_attention(T_FULL)
    in_maps = make_in_maps(x, W_qkv, b_qkv, W_pr)
    res = run_bass_kernel_spmd(_CACHE["nc"], in_maps, core_ids=list(range(8)))
    parts = [r["outT"] for r in res.results]
    return assemble_output(parts, b_pr)


# revision 4
# speedup vs baseline: 1.0312x; 1.0034x over previous
"""Causal self-attention (B=4, T=2048, C=1024, H=16) on 8 TRN2 NeuronCores.

Sharding: core = (batch b, head-group g) with b = core//2, g = core%2.
Each core computes, for its batch and its 8 heads:
  QKV projection (W_qkv column shard), causal attention, and a PARTIAL
  output projection (W_pr row shard).  Host sums the two partials per
  batch and adds b_pr.

v2 changes vs baseline (449us):
  - per-chunk K^T / V tiles (no cross-chunk WAR; sweeps of chunk j+1 can
    overlap attention of chunk j so the PE never idles / stays at max
    p-state)
  - diagonal k-tiles narrowed to valid columns (scores/exp/AV skip the
    fully-masked region; affine_select shrinks to the [2,128] diagonal
    sub-block, fused over the head pair via a 2-dim AP)
  - V bias via a precomputed broadcast tile + DVE add (was a PE matmul
    per t-tile)
  - softmax denominators: one fused [1,1024] Ln + Exp(-x) per head-pair
    (was per-head), Dinv broadcast by a rank-1 bf16 matmul into the
    UPPER rows (64:128) of the same yps PSUM bank (tile_position
    (64,64)), one fused [64,1024] PSUM->SBUF copy, then two DVE muls
  - PSUM: ps pool 2x1 bank (sweeps+proj), sc 2x2 banks, yps 1x2 banks
"""

import numpy as np

import concourse.bass as bass
import concourse.mybir as mybir
import concourse.tile as tile
from concourse.bass_utils import run_bass_kernel_spmd


def _split_multiwaits(nc: bass.Bass, max_waits: int = 1) -> None:
    """The walrus build in this container rejects >max_waits sync-waits on an
    instruction ("Too many sync wait commands").  Move extra waits onto
    same-engine NoOps inserted immediately before the instruction — the
    engine blocks on each NoOp's wait first, so semantics are unchanged."""
    n = 0
    for fn in nc.m.functions:
        for blk in fn.blocks:
            out = []
            for inst in blk.instructions:
                si = getattr(inst, "sync_info", None)
                waits = list(si.on_wait) if si is not None and si.on_wait else []
                if len(waits) > max_waits:
                    keep = waits[-max_waits:]
                    for w in waits[: -max_waits]:
                        nop = mybir.InstNoOp(name=f"{inst.name}-w{n}", ins=[], outs=[])
                        n += 1
                        nop.engine = inst.engine
                        nop.sync_info = mybir.SyncInfo(on_wait=[w], on_update=[])
                        out.append(nop)
                    inst.sync_info = mybir.SyncInfo(
                        on_wait=keep, on_update=list(si.on_update or [])
                    )
                out.append(inst)
            blk.instructions = out

AF = mybir.ActivationFunctionType
ALU = mybir.AluOpType

F32 = mybir.dt.float32
BF16 = mybir.dt.bfloat16

B, T_FULL, C = 4, 2048, 1024
H, HD = 16, 64
HPC = 8              # heads per core
GC = HPC * HD        # 512: per-core head-group width
P = 128
CH = 512             # q-chunk width
NKC = C // P         # 8 k-tiles over the C contraction
NTC = CH // P        # 4 t-tiles per chunk

NP_BF16 = mybir.dt.np(BF16)


def build_attention(
    T: int = T_FULL,
    split_waits: bool = True,
    filler_sweep: bool = True,
    filler_proj: bool = True,
    drain_ki: int = 1,
    drain_hp: int = 1,
    debug_taps: bool = False,
) -> bass.Bass:
    assert T % CH == 0
    nch = T // CH        # q-chunks

    nc = bass.Bass("TRN2", debug=False, num_devices=8)

    xT_d = nc.dram_tensor("xT", [C, T], BF16, kind="ExternalInput").ap()
    wq_d = nc.dram_tensor("wq", [C, GC], BF16, kind="ExternalInput").ap()
    wk_d = nc.dram_tensor("wk", [C, GC], BF16, kind="ExternalInput").ap()
    wv_d = nc.dram_tensor("wv", [C, GC], BF16, kind="ExternalInput").ap()
    bq_d = nc.dram_tensor("bq", [GC], F32, kind="ExternalInput").ap()
    bk_d = nc.dram_tensor("bk", [GC], F32, kind="ExternalInput").ap()
    bv_d = nc.dram_tensor("bv", [GC], BF16, kind="ExternalInput").ap()
    wpr_d = nc.dram_tensor("wpr", [GC, C], BF16, kind="ExternalInput").ap()
    out_d = nc.dram_tensor("outT", [C, T], F32, kind="ExternalOutput").ap()
    dbg = {}
    if debug_taps:
        for c in range(T // CH):
            dbg[f"qt{c}"] = nc.dram_tensor(f"dbg_qt{c}", [P, GC // P, CH], BF16, kind="ExternalOutput").ap()
            dbg[f"kt{c}"] = nc.dram_tensor(f"dbg_kt{c}", [P, GC // P, CH], BF16, kind="ExternalOutput").ap()
            dbg[f"v{c}"] = nc.dram_tensor(f"dbg_v{c}", [P, NTC, HPC, HD + 1], BF16, kind="ExternalOutput").ap()
            dbg[f"yt{c}"] = nc.dram_tensor(f"dbg_yt{c}", [P, GC // P, CH], BF16, kind="ExternalOutput").ap()

    with tile.TileContext(nc) as tc:
        with (
            tc.tile_pool(name="singles", bufs=1) as singles,
            tc.tile_pool(name="xt", bufs=2) as xt_pool,
            tc.tile_pool(name="qt", bufs=3) as qt_pool,
            tc.tile_pool(name="at", bufs=6) as at_pool,
            tc.tile_pool(name="yt", bufs=3) as yt_pool,
            tc.tile_pool(name="dd", bufs=2) as d_pool,
            tc.tile_pool(name="bc", bufs=2) as bc_pool,
            tc.tile_pool(name="ot", bufs=3) as out_pool,
            tc.tile_pool(name="ps", bufs=1, space="PSUM") as ps_pool,
            tc.tile_pool(name="bcps", bufs=1, space="PSUM") as bc_psum,
            tc.tile_pool(name="scps", bufs=2, space="PSUM") as sc_ps,
            tc.tile_pool(name="yps", bufs=1, space="PSUM") as y_ps,
        ):
            # ---- resident tensors ----
            # weights arrive in per-output-tile column chunks so the first
            # sweep matmuls can start after ~1/4 of wq instead of the full
            # 4MB of weights
            wq_sb = singles.tile([P, NKC, GC], BF16)
            wk_sb = singles.tile([P, NKC, GC], BF16)
            wv_sb = singles.tile([P, NKC, GC], BF16)
            xt0_t = xt_pool.tile([P, NKC, CH], BF16, tag="xt")
            for kc in range(NKC):
                nc.sync.dma_start(
                    out=xt0_t[:, kc, :], in_=xT_d[kc * P : (kc + 1) * P, 0:CH]
                )
            for m in range(GC // P):
                msl = slice(m * P, (m + 1) * P)
                nc.sync.dma_start(
                    out=wq_sb[:, :, msl],
                    in_=wq_d[:, msl].rearrange("(kc p) n -> p kc n", p=P),
                )
            for m in range(GC // P):
                msl = slice(m * P, (m + 1) * P)
                nc.sync.dma_start(
                    out=wk_sb[:, :, msl],
                    in_=wk_d[:, msl].rearrange("(kc p) n -> p kc n", p=P),
                )
            nc.sync.dma_start(out=wv_sb, in_=wv_d.rearrange("(kc p) n -> p kc n", p=P))
            wpr_sb = singles.tile([P, GC // P, C], BF16)
            nc.sync.dma_start(
                out=wpr_sb, in_=wpr_d.rearrange("(kp p) m -> p kp m", p=P)
            )
            bqk_sb = singles.tile([P, 2, GC // P], F32)
            nc.sync.dma_start(
                out=bqk_sb[:, 0, :], in_=bq_d.rearrange("(m p) -> p m", p=P)
            )
            nc.sync.dma_start(
                out=bqk_sb[:, 1, :], in_=bk_d.rearrange("(m p) -> p m", p=P)
            )
            bv_sb = singles.tile([1, GC], BF16)
            nc.sync.dma_start(out=bv_sb, in_=bv_d.rearrange("(o n) -> o n", o=1))
            ones_sb = singles.tile([1, P], BF16)
            nc.vector.memset(ones_sb, 1.0)
            ones64_sb = singles.tile([P, HD], BF16)
            nc.vector.memset(ones64_sb, 1.0)

            # V bias as a [128, GC] broadcast tile (built once via rank-1 mm)
            bvb_ps = ps_pool.tile([P, GC], F32, tag="ps")
            nc.tensor.matmul(bvb_ps, lhsT=ones_sb, rhs=bv_sb, start=True, stop=True)
            bvb_sb = singles.tile([P, GC], BF16)
            nc.vector.tensor_copy(bvb_sb, bvb_ps)

            # causal triangle mask for the [128,128] diagonal sub-block,
            # doubled along a middle dim so one DVE mul covers both heads
            # of a pair: keep (p, f) iff f >= p
            tri_sb = singles.tile([P, 2, P], BF16)
            nc.vector.memset(tri_sb, 1.0)
            nc.gpsimd.affine_select(
                out=tri_sb,
                in_=tri_sb,
                pattern=[[0, 2], [1, P]],
                compare_op=ALU.is_ge,
                fill=0.0,
                base=0,
                channel_multiplier=-1,
            )

            # per-chunk K^T and V tiles (separate tiles -> no cross-chunk WAR)
            kt_cs = [
                singles.tile([P, GC // P, CH], BF16, name=f"kt{c}") for c in range(nch)
            ]
            v_cs = [
                singles.tile([P, NTC, HPC, HD + 1], BF16, name=f"v{c}")
                for c in range(nch)
            ]
            for c in range(nch):
                nc.vector.memset(v_cs[c][:, :, :, HD : HD + 1], 1.0)

            # ---- filler queue: sweep/projection work emitted interleaved
            # with attention iterations so the PE instruction stream stays
            # dense (keeps the PE p-state ramped and fills exp-wait gaps) ----
            from collections import deque

            filler_q = deque()
            qts = {}
            counts = {"enq": 0, "dr": 0}
            need = {}

            def enq(fn):
                filler_q.append(fn)
                counts["enq"] += 1

            def drain1():
                if filler_q:
                    filler_q.popleft()()
                    counts["dr"] += 1

            def drain_all():
                while filler_q:
                    drain1()

            def enqueue_sweep(jj):
                """Queue QKV-sweep emission units for chunk jj (1 matmul or
                1 DVE add per unit). Issues the xt DMA immediately."""
                if jj == 0:
                    xt_t = xt0_t
                else:
                    xt_t = xt_pool.tile(
                        [P, NKC, CH], BF16, tag="xt", name=f"xt{jj}"
                    )

                    def xt_dma():
                        nc.sync.dma_start(
                            out=xt_t,
                            in_=xT_d[:, jj * CH : (jj + 1) * CH].rearrange(
                                "(kc p) t -> p kc t", p=P
                            ),
                        )

                    enq(xt_dma)
                qt_t = qt_pool.tile([P, GC // P, CH], BF16, tag="qt", name=f"qt{jj}")
                qts[jj] = qt_t
                box = {}

                def swps(nm):
                    if jj == 0 and swps.n % 2 == 1:
                        t = sc_ps.tile([P, CH], F32, tag="sc", name=nm)
                    else:
                        t = ps_pool.tile([P, CH], F32, tag="ps", name=nm)
                    swps.n += 1
                    return t

                swps.n = 0

                def q_mm(mq, kc):
                    if kc == 0:
                        box["ps"] = swps(f"psq{jj}_{mq}")
                    nc.tensor.matmul(
                        box["ps"],
                        lhsT=wq_sb[:, kc, mq * P : (mq + 1) * P],
                        rhs=xt_t[:, kc, :],
                        start=(kc == 0),
                        stop=(kc == NKC - 1),
                    )

                def q_add(mq):
                    nc.vector.tensor_scalar_add(
                        out=qt_t[:, mq, :],
                        in0=box["ps"],
                        scalar1=bqk_sb[:, 0, mq : mq + 1],
                    )

                def k_mm(mk, kc):
                    if kc == 0:
                        box["ps"] = swps(f"psk{jj}_{mk}")
                    nc.tensor.matmul(
                        box["ps"],
                        lhsT=wk_sb[:, kc, mk * P : (mk + 1) * P],
                        rhs=xt_t[:, kc, :],
                        start=(kc == 0),
                        stop=(kc == NKC - 1),
                    )

                def k_add(mk):
                    nc.vector.tensor_scalar_add(
                        out=kt_cs[jj][:, mk, :],
                        in0=box["ps"],
                        scalar1=bqk_sb[:, 1, mk : mk + 1],
                    )

                def v_mm(tl, kc):
                    if kc == 0:
                        box["ps"] = swps(f"psv{jj}_{tl}")
                    nc.tensor.matmul(
                        box["ps"],
                        lhsT=xt_t[:, kc, tl * P : (tl + 1) * P],
                        rhs=wv_sb[:, kc, :],
                        start=(kc == 0),
                        stop=(kc == NKC - 1),
                    )

                def v_add(tl):
                    nc.vector.tensor_add(
                        v_cs[jj][:, tl, :, 0:HD],
                        box["ps"].rearrange("p (h d) -> p h d", h=HPC),
                        bvb_sb.rearrange("p (h d) -> p h d", h=HPC),
                    )

                def q_tile(mq):
                    for kc in range(NKC):
                        q_mm(mq, kc)
                    q_add(mq)

                def k_tile(mk):
                    for kc in range(NKC):
                        k_mm(mk, kc)
                    k_add(mk)

                def v_tile(tl):
                    for kc in range(NKC):
                        v_mm(tl, kc)
                    v_add(tl)

                for mq in range(GC // P):
                    enq(lambda mq=mq: q_tile(mq))
                for mk in range(GC // P):
                    enq(lambda mk=mk: k_tile(mk))
                for tl in range(NTC):
                    enq(lambda tl=tl: v_tile(tl))

            def enqueue_proj(jj, yt_t):
                box = {}
                last = jj == nch - 1

                def ppsel(nm, m):
                    if last and m % 2 == 1:
                        return sc_ps.tile([P, CH], F32, tag="sc", name=nm)
                    return ps_pool.tile([P, CH], F32, tag="ps", name=nm)

                def p_mm(m, kp):
                    if kp == 0:
                        box["pp"] = ppsel(f"pp{jj}_{m}", m)
                    nc.tensor.matmul(
                        box["pp"],
                        lhsT=wpr_sb[:, kp, m * P : (m + 1) * P],
                        rhs=yt_t[:, kp, :],
                        start=(kp == 0),
                        stop=(kp == GC // P - 1),
                    )

                def p_out(m):
                    ot = out_pool.tile([P, CH], F32, tag="ot", name=f"ot{jj}_{m}")
                    nc.vector.tensor_copy(ot, box["pp"])
                    nc.sync.dma_start(
                        out=out_d[m * P : (m + 1) * P, jj * CH : (jj + 1) * CH],
                        in_=ot,
                    )

                def p_tile(m):
                    for kp in range(GC // P):
                        p_mm(m, kp)
                    p_out(m)

                for m in range(C // P):
                    enq(lambda m=m: p_tile(m))

            enqueue_sweep(0)
            drain_all()

            for j in range(nch):
                nkt = NTC * (j + 1)         # k-tiles valid for this q-chunk
                if j + 1 < nch:
                    enqueue_sweep(j + 1)
                    need[j + 1] = counts["enq"]
                    if not filler_sweep:
                        drain_all()
                # sweep j must be fully EMITTED before attention j reads it
                while counts["dr"] < need.get(j, 0):
                    drain1()
                # spread the present queue evenly over this chunk's iterations
                dr_base = counts["dr"]
                q_now = len(filler_q)
                iters = nkt * (HPC // 2)
                it_i = 0
                qt_t = qts[j]

                # ---- attention for chunk j ----
                yt_t = yt_pool.tile([P, GC // P, CH], BF16, tag="yt", name=f"yt{j}")
                for hp in range(HPC // 2):
                    yps = y_ps.tile([P, 2 * CH], F32, tag="yps", name=f"yps{j}_{hp}")
                    for ki in range(nkt):
                        c, tl = divmod(ki, NTC)
                        r = ki - NTC * j
                        off = 0 if r < 0 else P * r
                        sc = sc_ps.tile(
                            [P, 2 * CH], F32, tag="sc", name=f"sc{j}_{hp}_{ki}"
                        )
                        scv = sc.rearrange("p (u f) -> p u f", u=2)
                        for u in range(2):  # u = head parity; po = 64*u
                            po = HD * u
                            nc.tensor.matmul(
                                scv[:, u, off:],
                                lhsT=kt_cs[c][po : po + HD, hp, tl * P : (tl + 1) * P],
                                rhs=qt_t[po : po + HD, hp, off:],
                                start=True,
                                stop=True,
                            )
                        at_t = at_pool.tile([P, 2 * CH], BF16)
                        atv = at_t.rearrange("p (u f) -> p u f", u=2)
                        nc.scalar.activation(
                            atv[:, :, off:], scv[:, :, off:], AF.Exp, scale=0.125
                        )
                        if r >= 0:  # diagonal sub-block: zero where q < k
                            nc.vector.tensor_mul(
                                atv[:, :, off : off + P],
                                atv[:, :, off : off + P],
                                tri_sb,
                            )
                        for u in range(2):
                            nc.tensor.matmul(
                                yps[0 : HD + 1, u * CH + off : (u + 1) * CH],
                                lhsT=v_cs[c][:, tl, 2 * hp + u, :],
                                rhs=atv[:, u, off:],
                                start=(ki == 0),
                                stop=(ki == nkt - 1),
                                skip_group_check=True,
                            )
                        it_i += 1
                        while (
                            counts["dr"] - dr_base < (it_i * q_now) // iters
                            and filler_q
                        ):
                            drain1()
                    # ---- drain yps to SBUF at once (frees the PSUM slot for
                    # the next head pair); normalize off the critical path:
                    # Dinv = exp(-ln D) fused over the pair, broadcast via
                    # rank-1 bf16 matmuls into 1-bank ps tiles, then two DVE
                    # muls reading yraw (SBUF) x bc (PSUM) ----
                    yraw = bc_pool.tile([P, 2 * CH], BF16, tag="yraw")
                    nc.vector.tensor_copy(yraw[0:HD, :], yps[0:HD, :])
                    dr_t = d_pool.tile([P, 2 * CH], F32)
                    dr2_t = d_pool.tile([P, 2 * CH], BF16, tag="dr2")
                    nc.scalar.activation(
                        dr_t[HD : HD + 1, :], yps[HD : HD + 1, :], AF.Ln
                    )
                    nc.scalar.activation(
                        dr2_t[HD : HD + 1, :],
                        dr_t[HD : HD + 1, :],
                        AF.Exp,
                        scale=-1.0,
                    )
                    for u in range(2):
                        po = HD * u
                        bc_ps = bc_psum.tile([HD, CH], F32, tag="bcps")
                        nc.tensor.matmul(
                            bc_ps,
                            lhsT=ones64_sb[HD : HD + 1, :],
                            rhs=dr2_t[HD : HD + 1, u * CH : (u + 1) * CH],
                            start=True,
                            stop=True,
                            tile_position=(HD, 0),
                        )
                        nc.vector.tensor_mul(
                            yt_t[po : po + HD, hp, :],
                            yraw[0:HD, u * CH : (u + 1) * CH],
                            bc_ps,
                        )

                if debug_taps:
                    nc.sync.dma_start(out=dbg[f"qt{j}"], in_=qt_t)
                    nc.sync.dma_start(out=dbg[f"kt{j}"], in_=kt_cs[j])
                    nc.sync.dma_start(out=dbg[f"v{j}"], in_=v_cs[j])
                    nc.sync.dma_start(out=dbg[f"yt{j}"], in_=yt_t)

                # ---- partial output projection for chunk j, as filler ----
                enqueue_proj(j, yt_t)
                if not filler_proj:
                    drain_all()

            drain_all()

    if split_waits:  # breaks CoreSim's sem bookkeeping; needed for walrus
        _split_multiwaits(nc)
    return nc


def make_in_maps(x, W_qkv, b_qkv, W_pr):
    """Shard FULL inputs into the 8 per-core input dicts."""
    x = np.asarray(x, dtype=np.float32)
    W_qkv = np.asarray(W_qkv, dtype=np.float32)
    b_qkv = np.asarray(b_qkv, dtype=np.float32)
    W_pr = np.asarray(W_pr, dtype=np.float32)
    in_maps = []
    for core in range(8):
        b, g = divmod(core, 2)
        sl = slice(g * GC, (g + 1) * GC)
        in_maps.append(
            {
                "xT": np.ascontiguousarray(x[b].T).astype(NP_BF16),
                "wq": np.ascontiguousarray(W_qkv[:, 0 * C :][:, sl]).astype(NP_BF16),
                "wk": np.ascontiguousarray(W_qkv[:, 1 * C :][:, sl]).astype(NP_BF16),
                "wv": np.ascontiguousarray(W_qkv[:, 2 * C :][:, sl]).astype(NP_BF16),
                "bq": np.ascontiguousarray(b_qkv[0 * C :][sl]),
                "bk": np.ascontiguousarray(b_qkv[1 * C :][sl]),
                "bv": np.ascontiguousarray(b_qkv[2 * C :][sl]).astype(NP_BF16),
                "wpr": np.ascontiguousarray(W_pr[sl, :]).astype(NP_BF16),
            }
        )
    return in_maps


def assemble_output(parts, b_pr):
    """parts: 8 per-core outT [C, T] partials -> full [B, T, C] output."""
    b_pr = np.asarray(b_pr, dtype=np.float32)
    out = np.empty((B, T_FULL, C), dtype=np.float32)
    for b in range(B):
        out[b] = (parts[2 * b] + parts[2 * b + 1]).T + b_pr
    return out


_CACHE = {}


def kernel(x, W_qkv, b_qkv, W_pr, b_pr):
    if "nc" not in _CACHE:
        _CACHE["nc"] = build_attention(T_FULL)
    in_maps = make_in_maps(x, W_qkv, b_qkv, W_pr)
    res = run_bass_kernel_spmd(_CACHE["nc"], in_maps, core_ids=list(range(8)))
    parts = [r["outT"] for r in res.results]
    return assemble_output(parts, b_pr)
